# revision 26
# baseline (speedup 1.0000x reference)
"""KANConvTranspose2d forward on 8 Trainium2 NeuronCores.

Row-parallel over in_features (2304/8 = 288 per core). The KANLinear is
recast as one bf16 GEMM per core: the per-(feature, basis) activation
matrix A[(i,s), n] (8 uniform-grid cubic B-spline bases + SiLU as a 9th
"basis") against host-prescaled weights W[(i,s), o] packed K-major into
full 128-row chunks. Spline bases use the cardinal-B-spline identity
B_s(u) = M4((u - g[i,s])/h) with M4 evaluated by truncated powers
M4(t) = (t^3 - 4(t-1)+^3 + 6(t-2)+^3 - 4(t-3)+^3)/6 on t clamped to
[0,4] (clamping makes the j=4 term vanish and kills cancellation).
Each core DMAs ~24MB of bf16 weights (the modeled DMA floor), streams
them through PE accumulating [64, 4608] in PSUM, and writes a bf16
partial; the host sums the 8 partials in f32 and folds.
"""

import os
from concurrent.futures import ThreadPoolExecutor

import ml_dtypes
import numpy as np

import concourse.bacc as bacc
import concourse.mybir as mybir
import concourse.tile as tile
from concourse.bass_utils import run_bass_kernel_spmd

# module constants
CIN, COUT = 16, 8
HIN = WIN = 8
KK, ST, PD = 3, 2, 1
HOUT = WOUT = 16
OH_IN = OW_IN = 4
OH_OUT = OW_OUT = 8
IN_F = CIN * KK * KK * OH_IN * OW_IN        # 2304
OUT_F = COUT * KK * KK * OH_OUT * OW_OUT    # 4608
B = 64
NCORE = 8
IC = IN_F // NCORE                          # 288 in_features per core
NS = 8                                      # spline bases per feature
KSP = IC * NS                               # 2304 spline K-rows per core
KTOT = KSP + IC                             # 2592 with the SiLU rows
# K-chunks: 18 spline x128, then SiLU 128+128+32
CHUNKS = [(kt * 128, 128) for kt in range(20)] + [(2560, 32)]
NBLK = 12                                   # out_features in 12 blocks
BW = OUT_F // NBLK                          # 384 (fits one PSUM bank)
GW = 3                                      # spline chunks per ACT/DVE pass
B0 = float((1.0 / 6.0) ** (1.0 / 3.0))     # cbrt of |M4 coef|/6
B1 = float((4.0 / 6.0) ** (1.0 / 3.0))

F32 = mybir.dt.float32
F16 = mybir.dt.float16
BF16 = mybir.dt.bfloat16
BF = ml_dtypes.bfloat16

_CACHE = {}


def _build_bass():
    nc = bacc.Bacc("TRN2", target_bir_lowering=False, debug=False,
                   num_devices=NCORE)
    # tpk chunks 0..17: clamped spline t values; chunks 18..20: raw u rows
    t_d = nc.dram_tensor("tpk", [128, 21, B], F16, kind="ExternalInput")
    w_d = nc.dram_tensor("w", [KTOT, OUT_F], BF16, kind="ExternalInput")
    y_d = nc.dram_tensor("y", [B, OUT_F], BF16, kind="ExternalOutput")
    AF = mybir.ActivationFunctionType
    MUL = mybir.AluOpType.mult
    ADD = mybir.AluOpType.add
    SUB = mybir.AluOpType.subtract

    with tile.TileContext(nc) as tc:
        with (
            tc.tile_pool(name="inp", bufs=1) as ipool,
            tc.tile_pool(name="abuf", bufs=1) as apool,
            tc.tile_pool(name="tmp", bufs=2) as tpool,
            tc.tile_pool(name="wst", bufs=11) as wpool,
            tc.tile_pool(name="epi", bufs=1) as epool,
            tc.tile_pool(name="psum", bufs=1, space="PSUM") as pspool,
        ):
            # activation() wants non-zero Relu biases as resident const
            # APs; tile-pool tiles give exact memset->reader dependencies
            for val in (-B1, -2.0, -3.0 * B1):
                ct = ipool.tile([128, 1], F32, tag=f"c{val}")
                nc.gpsimd.memset(ct[:], val)
                nc.const_aps.aps[(F32, val)] = ct[:]

            # first weight chunk leads the DMA stream; t follows
            w_first = wpool.tile([128, OUT_F // 2], BF16, tag="w")
            nc.sync.dma_start(out=w_first[:], in_=w_d[0:128, 0:OUT_F // 2])
            t_sb = ipool.tile([128, 21, B], F16, tag="t")
            nc.sync.dma_start(out=t_sb[:, 0:2 * GW, :], in_=t_d[:, 0:2 * GW, :])

            # spline lhsT tiles: M4(t) = (b0·t)^3 + ((t-2)+)^3
            #                          - (b1·(t-1)+)^3 - (b1·(t-3)+)^3
            a_sp = []

            def spline_group(g):
                t_ap = t_sb[:, g * GW:(g + 1) * GW, :]
                P0 = tpool.tile([128, GW, B], F32, tag="p0")
                P1 = tpool.tile([128, GW, B], F32, tag="p1")
                P2 = tpool.tile([128, GW, B], F32, tag="p2")
                P3 = tpool.tile([128, GW, B], F32, tag="p3")
                q0 = tpool.tile([128, GW, B], F32, tag="q0")
                q1 = tpool.tile([128, GW, B], F32, tag="q1")
                q2 = tpool.tile([128, GW, B], F32, tag="q2")
                q3 = tpool.tile([128, GW, B], F32, tag="q3")
                nc.scalar.activation(P0[:], t_ap, AF.Copy, scale=B0)
                nc.scalar.activation(P1[:], t_ap, AF.Relu, -B1, B1)
                nc.scalar.activation(P2[:], t_ap, AF.Relu, -2.0)
                nc.scalar.activation(P3[:], t_ap, AF.Relu, -3.0 * B1, B1)
                nc.scalar.activation(q0[:], P0[:], AF.Square)
                nc.scalar.activation(q2[:], P2[:], AF.Square)
                nc.vector.tensor_tensor(out=q1[:], in0=P1[:], in1=P1[:], op=MUL)
                nc.vector.tensor_tensor(out=q3[:], in0=P3[:], in1=P3[:], op=MUL)
                nc.vector.tensor_tensor(out=q0[:], in0=q0[:], in1=P0[:], op=MUL)
                nc.vector.tensor_tensor(out=q1[:], in0=q1[:], in1=P1[:], op=MUL)
                nc.vector.tensor_tensor(out=q2[:], in0=q2[:], in1=P2[:], op=MUL)
                nc.vector.tensor_tensor(out=q3[:], in0=q3[:], in1=P3[:], op=MUL)
                nc.vector.tensor_tensor(out=q0[:], in0=q0[:], in1=q2[:], op=ADD)
                nc.vector.tensor_tensor(out=q1[:], in0=q1[:], in1=q3[:], op=ADD)
                ag = apool.tile([128, GW, B], BF16, tag=f"a{g}")
                nc.vector.tensor_tensor(out=ag[:], in0=q0[:], in1=q1[:], op=SUB)
                a_sp.append(ag)

            spline_group(0)
            spline_group(1)
            nc.sync.dma_start(out=t_sb[:, 2 * GW:21, :],
                              in_=t_d[:, 2 * GW:21, :])
            for g in range(2, 18 // GW):
                spline_group(g)

            # SiLU lhsT tiles (base path rides as bases 18..20)
            a_silu = apool.tile([128, 3, B], BF16, tag="asilu")
            nc.scalar.activation(a_silu[:], t_sb[:, 18:21, :], AF.Silu)

            # weight stream + matmul accumulation, in two out-feature
            # phases so phase A's eviction and output DMA overlap with
            # phase B's stream (halves the post-stream tail)
            ps = [pspool.tile([128, BW], F32, tag=f"ps{b}", name=f"ps{b}")
                  for b in range(6)]
            nkt = len(CHUNKS)
            HOF = OUT_F // 2
            for ph in range(2):
                for kt, (off, p) in enumerate(CHUNKS):
                    if kt < 18:
                        g, r = divmod(kt, GW)
                        lhsT = a_sp[g][:, r, :]
                    else:
                        lhsT = a_silu[:p, kt - 18, :]
                    # N-split the stream-final chunks so their matmuls
                    # chase progressively narrower DMAs (the post-stream
                    # tail then hangs off a single-bank segment)
                    segs = [(0, 6)] if not (ph == 1 and kt >= nkt - 2) \
                        else [(0, 3), (3, 6)]
                    for b0_, b1_ in segs:
                        nb = b1_ - b0_
                        c0 = ph * HOF + b0_ * BW
                        if ph == 0 and kt == 0 and b0_ == 0:
                            w_sb = w_first
                        else:
                            w_sb = wpool.tile([p, nb * BW], BF16, tag="w")
                            nc.sync.dma_start(
                                out=w_sb[:],
                                in_=w_d[off:off + p, c0:c0 + nb * BW])
                        for j in range(nb):
                            nc.tensor.matmul(
                                ps[b0_ + j][ph * B:(ph + 1) * B, :],
                                lhsT, w_sb[:, j * BW:(j + 1) * BW],
                                start=(kt == 0), stop=(kt == nkt - 1),
                                tile_position=(0, 64 * ph))
                # evict this phase's PSUM rows to SBUF right away (ACT/DVE
                # are idle mid-stream); phase A's output DMA is deferred
                # past the weight stream into the tail's idle DMA window
                y_sb = epool.tile([B, 6, BW], BF16, tag=f"ysb{ph}")
                for bank in range(6):
                    src = ps[bank][ph * B:(ph + 1) * B, :]
                    dst = y_sb[:, bank, :]
                    if bank % 2 == 0:
                        nc.scalar.copy(dst, src)
                    else:
                        nc.vector.tensor_copy(out=dst, in_=src)
                if ph == 0:
                    y_sb_a = y_sb
                else:
                    # negative offset = appears later to the scheduler, so
                    # this DMA stays behind the weight stream and lands in
                    # the tail's idle DMA window
                    with tc.high_priority(offset=-(1 << 20)):
                        nc.sync.dma_start(out=y_d[:, 0:HOF], in_=y_sb_a[:])
                    # split so the last transfer is short (final sem waits
                    # only on a half-size DMA)
                    nc.sync.dma_start(
                        out=y_d[:, HOF:HOF + 3 * BW], in_=y_sb[:, 0:3, :])
                    nc.sync.dma_start(
                        out=y_d[:, HOF + 3 * BW:], in_=y_sb[:, 3:6, :])

    nc.compile()
    return nc


def _get_nc():
    if "nc" not in _CACHE:
        _CACHE["nc"] = _build_bass()
    return _CACHE["nc"]


def _unfold(x):
    xp = np.pad(x, ((0, 0), (0, 0), (PD, PD), (PD, PD)))
    pats = np.stack(
        [xp[:, :, i:i + (OH_IN - 1) * ST + 1:ST, j:j + (OW_IN - 1) * ST + 1:ST]
         for i in range(KK) for j in range(KK)], axis=2)
    return pats.reshape(B, CIN * KK * KK, OH_IN * OW_IN).reshape(B, IN_F)


def _fold(y):
    # y: [B, OUT_F] -> scatter-add -> [B, COUT, 16, 16]
    u6 = y.reshape(B, COUT, KK, KK, OH_OUT, OW_OUT)
    out = np.zeros((B, COUT, HOUT + 2, WOUT + 2), np.float32)
    for i in range(KK):
        for j in range(KK):
            out[:, :, i:i + 2 * OH_OUT:2, j:j + 2 * OW_OUT:2] += u6[:, :, i, j]
    return np.ascontiguousarray(out[:, :, PD:HOUT + PD, PD:WOUT + PD])


def kernel(x, base_weight, spline_weight, spline_scaler, grid):
    nc = _get_nc()

    u = _unfold(np.asarray(x, np.float32))                   # [B, IN_F]
    uT = np.ascontiguousarray(u.T)                           # [IN_F, B]
    g = np.asarray(grid, np.float32)
    h = (g[:, 1] - g[:, 0]).astype(np.float32)
    t = (uT[:, None, :] - g[:, :NS, None]) / h[:, None, None]
    np.clip(t, 0.0, 4.0, out=t)                              # [IN_F, NS, B]
    t = np.ascontiguousarray(t.reshape(NCORE, 18, 128, B))

    sw = np.asarray(spline_weight, np.float32)
    sc = np.asarray(spline_scaler, np.float32)
    bw = np.asarray(base_weight, np.float32)

    def prep_core(c):
        r0, r1 = c * IC, (c + 1) * IC
        tpk = np.empty((128, 21, B), np.float16)
        tpk[:, :18] = t[c].transpose(1, 0, 2)
        up = np.zeros((3, 128, B), np.float32)
        up.reshape(3 * 128, B)[:IC] = uT[r0:r1]
        tpk[:, 18:] = up.transpose(1, 0, 2)
        blk = sw[:, r0:r1, :] * sc[:, r0:r1, None]           # [OUT_F, IC, NS]
        wf = np.empty((KTOT, OUT_F), BF)
        wf[:KSP] = blk.reshape(OUT_F, KSP).T.astype(BF)
        wf[KSP:] = bw[:, r0:r1].T.astype(BF)
        return {"tpk": tpk, "w": wf}

    with ThreadPoolExecutor(NCORE) as ex:
        in_maps = list(ex.map(prep_core, range(NCORE)))

    res = run_bass_kernel_spmd(nc, in_maps, list(range(NCORE)))
    y = np.zeros((B, OUT_F), np.float32)
    for c in range(NCORE):
        y += res.results[c]["y"].astype(np.float32)
    return _fold(y)


def _warmup():
    # Compile (and touch the devices) at import so the first kernel()
    # call doesn't pay the Bass build + neuronxcc compile latency.
    try:
        nc = _get_nc()
        zin = {
            "tpk": np.zeros((128, 21, B), np.float16),
            "w": np.zeros((KTOT, OUT_F), BF),
        }
        run_bass_kernel_spmd(nc, [dict(zin) for _ in range(NCORE)],
                             list(range(NCORE)))
    except Exception:
        pass


if not os.environ.get("KERNEL_NO_WARMUP"):
    _warmup()


# revision 27
# speedup vs baseline: 1.0109x; 1.0109x over previous
"""KANConvTranspose2d forward on 8 Trainium2 NeuronCores.

Row-parallel over in_features (2304/8 = 288 per core). The KANLinear is
recast as one bf16 GEMM per core: the per-(feature, basis) activation
matrix A[(i,s), n] (8 uniform-grid cubic B-spline bases + SiLU as a 9th
"basis") against host-prescaled weights W[(i,s), o] packed K-major into
full 128-row chunks. Spline bases use the cardinal-B-spline identity
B_s(u) = M4((u - g[i,s])/h) with M4 evaluated by truncated powers
M4(t) = (t^3 - 4(t-1)+^3 + 6(t-2)+^3 - 4(t-3)+^3)/6 on t clamped to
[0,4] (clamping makes the j=4 term vanish and kills cancellation).
Each core DMAs ~24MB of bf16 weights (the modeled DMA floor), streams
them through PE accumulating [64, 4608] in PSUM, and writes a bf16
partial; the host sums the 8 partials in f32 and folds.
"""

import os
from concurrent.futures import ThreadPoolExecutor

import ml_dtypes
import numpy as np

import concourse.bacc as bacc
import concourse.mybir as mybir
import concourse.tile as tile
from concourse.bass_utils import run_bass_kernel_spmd

# module constants
CIN, COUT = 16, 8
HIN = WIN = 8
KK, ST, PD = 3, 2, 1
HOUT = WOUT = 16
OH_IN = OW_IN = 4
OH_OUT = OW_OUT = 8
IN_F = CIN * KK * KK * OH_IN * OW_IN        # 2304
OUT_F = COUT * KK * KK * OH_OUT * OW_OUT    # 4608
B = 64
NCORE = 8
IC = IN_F // NCORE                          # 288 in_features per core
NS = 8                                      # spline bases per feature
KSP = IC * NS                               # 2304 spline K-rows per core
KTOT = KSP + IC                             # 2592 with the SiLU rows
# K-chunks: 18 spline x128, then SiLU 128+128+32
CHUNKS = [(kt * 128, 128) for kt in range(20)] + [(2560, 32)]
NBLK = 12                                   # out_features in 12 blocks
BW = OUT_F // NBLK                          # 384 (fits one PSUM bank)
GW = 3                                      # spline chunks per ACT/DVE pass
B0 = float((1.0 / 6.0) ** (1.0 / 3.0))     # cbrt of |M4 coef|/6
B1 = float((4.0 / 6.0) ** (1.0 / 3.0))

F32 = mybir.dt.float32
F16 = mybir.dt.float16
BF16 = mybir.dt.bfloat16
BF = ml_dtypes.bfloat16

_CACHE = {}


def _build_bass():
    nc = bacc.Bacc("TRN2", target_bir_lowering=False, debug=False,
                   num_devices=NCORE)
    # tpk chunks 0..17: clamped spline t values; chunks 18..20: raw u rows
    t_d = nc.dram_tensor("tpk", [128, 21, B], F16, kind="ExternalInput")
    w_d = nc.dram_tensor("w", [KTOT, OUT_F], BF16, kind="ExternalInput")
    y_d = nc.dram_tensor("y", [B, OUT_F], BF16, kind="ExternalOutput")
    AF = mybir.ActivationFunctionType
    MUL = mybir.AluOpType.mult
    ADD = mybir.AluOpType.add
    SUB = mybir.AluOpType.subtract

    with tile.TileContext(nc) as tc:
        with (
            tc.tile_pool(name="inp", bufs=1) as ipool,
            tc.tile_pool(name="abuf", bufs=1) as apool,
            tc.tile_pool(name="tmp", bufs=2) as tpool,
            tc.tile_pool(name="wst", bufs=11) as wpool,
            tc.tile_pool(name="epi", bufs=1) as epool,
            tc.tile_pool(name="psum", bufs=1, space="PSUM") as pspool,
        ):
            # activation() wants non-zero Relu biases as resident const
            # APs; tile-pool tiles give exact memset->reader dependencies
            for val in (-B1, -2.0, -3.0 * B1):
                ct = ipool.tile([128, 1], F32, tag=f"c{val}")
                nc.gpsimd.memset(ct[:], val)
                nc.const_aps.aps[(F32, val)] = ct[:]

            # first weight chunk leads the DMA stream; t follows
            w_first = wpool.tile([128, OUT_F // 2], BF16, tag="w")
            nc.sync.dma_start(out=w_first[:], in_=w_d[0:128, 0:OUT_F // 2])
            t_sb = ipool.tile([128, 21, B], F16, tag="t")
            nc.sync.dma_start(out=t_sb[:, 0:2 * GW, :], in_=t_d[:, 0:2 * GW, :])

            # spline lhsT tiles: M4(t) = (b0·t)^3 + ((t-2)+)^3
            #                          - (b1·(t-1)+)^3 - (b1·(t-3)+)^3
            a_sp = []

            def spline_group(g):
                t_ap = t_sb[:, g * GW:(g + 1) * GW, :]
                P0 = tpool.tile([128, GW, B], F32, tag="p0")
                P1 = tpool.tile([128, GW, B], F32, tag="p1")
                P2 = tpool.tile([128, GW, B], F32, tag="p2")
                P3 = tpool.tile([128, GW, B], F32, tag="p3")
                q0 = tpool.tile([128, GW, B], F32, tag="q0")
                q1 = tpool.tile([128, GW, B], F32, tag="q1")
                q2 = tpool.tile([128, GW, B], F32, tag="q2")
                q3 = tpool.tile([128, GW, B], F32, tag="q3")
                nc.scalar.activation(P0[:], t_ap, AF.Copy, scale=B0)
                nc.scalar.activation(P1[:], t_ap, AF.Relu, -B1, B1)
                nc.scalar.activation(P2[:], t_ap, AF.Relu, -2.0)
                nc.scalar.activation(P3[:], t_ap, AF.Relu, -3.0 * B1, B1)
                nc.scalar.activation(q0[:], P0[:], AF.Square)
                nc.scalar.activation(q2[:], P2[:], AF.Square)
                nc.vector.tensor_tensor(out=q1[:], in0=P1[:], in1=P1[:], op=MUL)
                nc.vector.tensor_tensor(out=q3[:], in0=P3[:], in1=P3[:], op=MUL)
                nc.vector.tensor_tensor(out=q0[:], in0=q0[:], in1=P0[:], op=MUL)
                nc.vector.tensor_tensor(out=q1[:], in0=q1[:], in1=P1[:], op=MUL)
                nc.vector.tensor_tensor(out=q2[:], in0=q2[:], in1=P2[:], op=MUL)
                nc.vector.tensor_tensor(out=q3[:], in0=q3[:], in1=P3[:], op=MUL)
                nc.vector.tensor_tensor(out=q0[:], in0=q0[:], in1=q2[:], op=ADD)
                nc.vector.tensor_tensor(out=q1[:], in0=q1[:], in1=q3[:], op=ADD)
                ag = apool.tile([128, GW, B], BF16, tag=f"a{g}")
                nc.vector.tensor_tensor(out=ag[:], in0=q0[:], in1=q1[:], op=SUB)
                a_sp.append(ag)

            spline_group(0)
            spline_group(1)
            nc.sync.dma_start(out=t_sb[:, 2 * GW:21, :],
                              in_=t_d[:, 2 * GW:21, :])
            for g in range(2, 18 // GW):
                spline_group(g)

            # SiLU lhsT tiles (base path rides as bases 18..20)
            a_silu = apool.tile([128, 3, B], BF16, tag="asilu")
            nc.scalar.activation(a_silu[:], t_sb[:, 18:21, :], AF.Silu)

            # weight stream + matmul accumulation, in two out-feature
            # phases so phase A's eviction and output DMA overlap with
            # phase B's stream (halves the post-stream tail)
            ps = [pspool.tile([128, BW], F32, tag=f"ps{b}", name=f"ps{b}")
                  for b in range(6)]
            nkt = len(CHUNKS)
            HOF = OUT_F // 2
            for ph in range(2):
                chunk_iter = list(enumerate(CHUNKS))
                if ph == 1:
                    # iterate the small SiLU chunk first so the stream
                    # ends on full chunks whose matmuls chase half-size
                    # DMAs closely (shorter post-stream tail)
                    chunk_iter = chunk_iter[-1:] + chunk_iter[:-1]
                for it, (kt, (off, p)) in enumerate(chunk_iter):
                    if kt < 18:
                        g, r = divmod(kt, GW)
                        lhsT = a_sp[g][:, r, :]
                    else:
                        lhsT = a_silu[:p, kt - 18, :]
                    # N-split the stream-final chunks so their matmuls
                    # chase the half-size DMAs
                    segs = [(0, 6)] if not (ph == 1 and it >= nkt - 2) \
                        else [(0, 3), (3, 6)]
                    for b0_, b1_ in segs:
                        nb = b1_ - b0_
                        c0 = ph * HOF + b0_ * BW
                        if ph == 0 and kt == 0 and b0_ == 0:
                            w_sb = w_first
                        else:
                            w_sb = wpool.tile([p, nb * BW], BF16, tag="w")
                            nc.sync.dma_start(
                                out=w_sb[:],
                                in_=w_d[off:off + p, c0:c0 + nb * BW])
                        for j in range(nb):
                            nc.tensor.matmul(
                                ps[b0_ + j][ph * B:(ph + 1) * B, :],
                                lhsT, w_sb[:, j * BW:(j + 1) * BW],
                                start=(it == 0), stop=(it == nkt - 1),
                                tile_position=(0, 64 * ph))
                # evict this phase's PSUM rows to SBUF right away (ACT/DVE
                # are idle mid-stream); phase A's output DMA is deferred
                # past the weight stream into the tail's idle DMA window
                y_sb = epool.tile([B, 6, BW], BF16, tag=f"ysb{ph}")
                for bank in range(6):
                    src = ps[bank][ph * B:(ph + 1) * B, :]
                    dst = y_sb[:, bank, :]
                    if bank % 2 == 0:
                        nc.scalar.copy(dst, src)
                    else:
                        nc.vector.tensor_copy(out=dst, in_=src)
                if ph == 0:
                    y_sb_a = y_sb
                else:
                    # negative offset = appears later to the scheduler, so
                    # this DMA stays behind the weight stream and lands in
                    # the tail's idle DMA window
                    with tc.high_priority(offset=-(1 << 20)):
                        nc.sync.dma_start(out=y_d[:, 0:HOF], in_=y_sb_a[:])
                    # split so the last transfer is short (final sem waits
                    # only on a half-size DMA)
                    nc.sync.dma_start(
                        out=y_d[:, HOF:HOF + 3 * BW], in_=y_sb[:, 0:3, :])
                    nc.sync.dma_start(
                        out=y_d[:, HOF + 3 * BW:], in_=y_sb[:, 3:6, :])

    nc.compile()
    return nc


def _get_nc():
    if "nc" not in _CACHE:
        _CACHE["nc"] = _build_bass()
    return _CACHE["nc"]


def _unfold(x):
    xp = np.pad(x, ((0, 0), (0, 0), (PD, PD), (PD, PD)))
    pats = np.stack(
        [xp[:, :, i:i + (OH_IN - 1) * ST + 1:ST, j:j + (OW_IN - 1) * ST + 1:ST]
         for i in range(KK) for j in range(KK)], axis=2)
    return pats.reshape(B, CIN * KK * KK, OH_IN * OW_IN).reshape(B, IN_F)


def _fold(y):
    # y: [B, OUT_F] -> scatter-add -> [B, COUT, 16, 16]
    u6 = y.reshape(B, COUT, KK, KK, OH_OUT, OW_OUT)
    out = np.zeros((B, COUT, HOUT + 2, WOUT + 2), np.float32)
    for i in range(KK):
        for j in range(KK):
            out[:, :, i:i + 2 * OH_OUT:2, j:j + 2 * OW_OUT:2] += u6[:, :, i, j]
    return np.ascontiguousarray(out[:, :, PD:HOUT + PD, PD:WOUT + PD])


def kernel(x, base_weight, spline_weight, spline_scaler, grid):
    nc = _get_nc()

    u = _unfold(np.asarray(x, np.float32))                   # [B, IN_F]
    uT = np.ascontiguousarray(u.T)                           # [IN_F, B]
    g = np.asarray(grid, np.float32)
    h = (g[:, 1] - g[:, 0]).astype(np.float32)
    t = (uT[:, None, :] - g[:, :NS, None]) / h[:, None, None]
    np.clip(t, 0.0, 4.0, out=t)                              # [IN_F, NS, B]
    t = np.ascontiguousarray(t.reshape(NCORE, 18, 128, B))

    sw = np.asarray(spline_weight, np.float32)
    sc = np.asarray(spline_scaler, np.float32)
    bw = np.asarray(base_weight, np.float32)

    def prep_core(c):
        r0, r1 = c * IC, (c + 1) * IC
        tpk = np.empty((128, 21, B), np.float16)
        tpk[:, :18] = t[c].transpose(1, 0, 2)
        up = np.zeros((3, 128, B), np.float32)
        up.reshape(3 * 128, B)[:IC] = uT[r0:r1]
        tpk[:, 18:] = up.transpose(1, 0, 2)
        blk = sw[:, r0:r1, :] * sc[:, r0:r1, None]           # [OUT_F, IC, NS]
        wf = np.empty((KTOT, OUT_F), BF)
        wf[:KSP] = blk.reshape(OUT_F, KSP).T.astype(BF)
        wf[KSP:] = bw[:, r0:r1].T.astype(BF)
        return {"tpk": tpk, "w": wf}

    with ThreadPoolExecutor(NCORE) as ex:
        in_maps = list(ex.map(prep_core, range(NCORE)))

    res = run_bass_kernel_spmd(nc, in_maps, list(range(NCORE)))
    y = np.zeros((B, OUT_F), np.float32)
    for c in range(NCORE):
        y += res.results[c]["y"].astype(np.float32)
    return _fold(y)


def _warmup():
    # Compile (and touch the devices) at import so the first kernel()
    # call doesn't pay the Bass build + neuronxcc compile latency.
    try:
        nc = _get_nc()
        zin = {
            "tpk": np.zeros((128, 21, B), np.float16),
            "w": np.zeros((KTOT, OUT_F), BF),
        }
        run_bass_kernel_spmd(nc, [dict(zin) for _ in range(NCORE)],
                             list(range(NCORE)))
    except Exception:
        pass


if not os.environ.get("KERNEL_NO_WARMUP"):
    _warmup()
